# revision 27
# baseline (speedup 1.0000x reference)
"""MoE top-1 routing kernel for Trainium2 (8 NeuronCores).

Reference computation (B=8, S=1024, D=768, E=8, F=3072):
    gates = softmax(x @ gate_w + gate_b); expert_idx = argmax(gates)
    out[t] = gelu(x[t] @ w1[e] + b1[e]) @ w2[e] + b2[e]   for e = expert_idx[t]
    (no gate-probability scaling)

Strategy (v5 — feature-sliced expert replication, tile-packed IO):
  * Routing on host in fp64 (softmax is monotonic, so argmax of logits ==
    argmax of gates).
  * Every core holds ALL 8 experts' weights for its own 1/8 slice of the
    F dimension (384 features) and processes ALL T=8192 tokens, grouped
    into 8 expert blocks with EXACT token counts — zero padding, perfectly
    balanced across cores.  Each core emits a partial FFN2 sum; the host
    adds the 8 partials + b2.
  * Tokens stream through SBUF in tiles of <=512 (one PSUM bank).  x and
    y use a TILE-PACKED DRAM layout: tile i's [KD, nt] block is stored
    contiguously per partition, so each tile transfer is a single ~6KB
    descriptor per partition.  (DMA rings pay ~44ns fixed per descriptor;
    1KB-line layouts double total ring time and starve the PE.)
  * Weight loads are paced into the emission stream ~2 blocks ahead of
    use — ring FIFO means issue order is priority order.
  * Matmuls in bf16 with fp32 PSUM accumulation; activations stay
    transposed ([feature, token]).  gelu (erf) on the Scalar engine with
    the b1 bias fused; FFN2 partials copied PSUM->SBUF as bf16 on the
    Vector engine and DMA'd out per tile.
  * PE warmup matmuls flip the HAM clock gate to 2.4 GHz while the head
    DMAs stream in.
"""

import sys

try:
    import concourse  # noqa: F401
except ImportError:
    sys.path.insert(0, "/opt/trn_rl_repo")

import numpy as np
import ml_dtypes

import concourse.bass as bass  # noqa: F401
import concourse.tile as tile
import concourse.mybir as mybir
from concourse import bacc
from concourse import bass_utils

BF16 = mybir.dt.bfloat16
F32 = mybir.dt.float32
AF = mybir.ActivationFunctionType

B, S, D, E = 8, 1024, 768, 8
F = 4 * D           # 3072
T = B * S           # 8192
N_CORES = 8
FS = F // N_CORES   # 384 features per core
KD = D // 128       # 6 contraction chunks over D (FFN1)
M1 = FS // 128      # 3 output chunks over the F-slice (FFN1)
K2 = FS // 128      # 3 contraction chunks over the F-slice (FFN2)
MD = D // 128       # 6 output chunks over D (FFN2)
MAX_N = 512         # moving-dim tile (one fp32 PSUM bank)
N_WARMUP = 45       # PE warmup matmuls: ~4.8us cold > one HAM window,
                    # sized to end right as the first tile's DMAs land so
                    # the clock flips to 2.4GHz before the real stream.

# Debug/profiling knobs (used by the local test harness only).
TRACE = False
LAST_RESULT = None


def _even_split(cap):
    """ceil(cap/512) near-equal tiles."""
    if cap <= 0:
        return []
    n = -(-cap // MAX_N)
    base, rem = divmod(cap, n)
    out, off = [], 0
    for i in range(n):
        sz = base + (1 if i < rem else 0)
        out.append((off, sz))
        off += sz
    return out


def _ramp_split(cap):
    """Small leading tiles so the first matmuls need little DMA."""
    lead = [64, 192, 256]
    out, off = [], 0
    for w in lead:
        if cap - off <= w + MAX_N:
            break
        out.append((off, w))
        off += w
    return out + [(off + o, w) for (o, w) in _even_split(cap - off)]


def _tail_split(cap):
    """Equal tiles, then descending small final tiles so the trailing
    y-out DMAs overlap compute and the final drain is tiny."""
    tail = [288, 160, 96]
    if cap <= sum(tail) + MAX_N:
        return _even_split(cap)
    out = _even_split(cap - sum(tail))
    off = cap - sum(tail)
    for w in tail:
        out.append((off, w))
        off += w
    return out


def make_sched(counts):
    """counts: per-slot token counts (schedule order).  Returns
    [(slot, n0, nt)] tile schedule over the concatenated token buffer."""
    sched = []
    off = 0
    nb = len(counts)
    for b, c in enumerate(counts):
        if c == 0:
            continue
        if b == nb - 1:
            tiles = _tail_split(c)
        else:
            tiles = _even_split(c)
        for (o, w) in tiles:
            sched.append((b, off + o, w))
        off += c
    return sched


def build_program(counts):
    counts = list(counts)
    sched = make_sched(counts)
    XL = KD * T          # packed x/y length per partition (elements)

    nc = bacc.Bacc("TRN2", target_bir_lowering=False, debug=False,
                   num_devices=N_CORES)

    xT_d = nc.dram_tensor("xT", (128, XL), BF16, kind="ExternalInput")
    w1_d = nc.dram_tensor("w1", (128, E, M1, KD, 128), BF16,
                          kind="ExternalInput")
    w2_d = nc.dram_tensor("w2", (128, E, MD, K2, 128), BF16,
                          kind="ExternalInput")
    b1_d = nc.dram_tensor("b1", (128, E, M1), F32, kind="ExternalInput")
    yT_d = nc.dram_tensor("yT", (128, XL), BF16, kind="ExternalOutput")

    NXB = 5  # xT streaming buffers

    with tile.TileContext(nc) as tc:
        with (
            tc.tile_pool(name="wts", bufs=1) as wts,
            tc.tile_pool(name="xb", bufs=NXB) as xbp,
            tc.tile_pool(name="act", bufs=4) as actp,
            tc.tile_pool(name="ps1", bufs=3, space="PSUM") as ps1,
            tc.tile_pool(name="ps2", bufs=5, space="PSUM") as ps2,
        ):
            w1 = wts.tile([128, E, M1, KD, 128], BF16, tag="w1")
            w2 = wts.tile([128, E, MD, K2, 128], BF16, tag="w2")
            b1 = wts.tile([128, E, M1], F32, tag="b1")
            warm = wts.tile([128, 128], BF16, tag="warm")
            nc.gpsimd.memset(warm[:], 0.0)
            wps = ps1.tile([128, 128], F32, tag="ps1",
                           padded_shape=[128, MAX_N])

            # PE warmup: dummy matmuls run while the head DMAs stream in,
            # flipping the HAM clock gate to 2.4 GHz before the real
            # matmul stream starts.
            for _ in range(N_WARMUP):
                nc.tensor.matmul(wps[:, :], warm[:, :], warm[:, :])

            # ---- packed x tile streaming ----
            # Tile i's tokens live at xT_d[:, xoff[i] : xoff[i]+KD*nt]
            # (chunk k at sub-offset k*nt).  One descriptor per partition.
            xoff = []
            o = 0
            for (_, _, nt) in sched:
                xoff.append(o)
                o += KD * nt

            xtiles = {}

            def xdma(i):
                _, _, nt = sched[i]
                xt = xbp.tile([128, KD * nt], BF16, tag="x",
                              padded_shape=[128, KD * MAX_N])
                xtiles[i] = xt
                nc.sync.dma_start(xt[:, :], xT_d[:, xoff[i]:xoff[i] + KD * nt])

            # ---- head DMAs ----
            # Only what the ramp tiles need right away.  Issue order IS
            # ring priority, so nothing bulky goes ahead of the first
            # tiles' dependencies.  The scalar queue (gelu) gets only two
            # small issues; paced weight loads ride on gpsimd (the y-out
            # queue) where act bufs=3 gives two tiles of slack.
            b0 = sched[0][0]
            xdma(0)
            nc.scalar.dma_start(w1[:, b0, 0, :, :], w1_d[:, b0, 0, :, :])
            nc.scalar.dma_start(b1[:], b1_d[:])
            nc.gpsimd.dma_start(w1[:, b0, 1, :, :], w1_d[:, b0, 1, :, :])
            nc.scalar.dma_start(w1[:, b0, 2, :, :], w1_d[:, b0, 2, :, :])
            xdma(1)
            nc.gpsimd.dma_start(w2[:, b0, 0:3, :, :], w2_d[:, b0, 0:3, :, :])
            nc.gpsimd.dma_start(w2[:, b0, 3:, :, :], w2_d[:, b0, 3:, :, :])
            xdma(2)
            blocks = []
            for (b, _, _) in sched:
                if b not in blocks:
                    blocks.append(b)

            def wdma(bi, part):
                """Paced weight load for the bi-th block in schedule order.
                part 0 = w1, part 1 = w2 (staggered by one tile to smooth
                ring pressure)."""
                if bi < len(blocks):
                    b = blocks[bi]
                    if part == 0:
                        nc.gpsimd.dma_start(w1[:, b, :, :, :],
                                            w1_d[:, b, :, :, :])
                    else:
                        nc.gpsimd.dma_start(w2[:, b, :, :, :],
                                            w2_d[:, b, :, :, :])

            def ffn1(i):
                b, n0, nt = sched[i]
                xt = xtiles.pop(i)
                h = actp.tile([128, K2, nt], BF16, tag="h",
                              padded_shape=[128, K2, MAX_N])
                for m in range(M1):
                    ps = ps1.tile([128, nt], F32, tag="ps1",
                                  padded_shape=[128, MAX_N])
                    for k in range(KD):
                        nc.tensor.matmul(
                            ps[:, :],
                            w1[:, b, m, k, :],
                            xt[:, k * nt:(k + 1) * nt],
                            start=(k == 0),
                            stop=(k == KD - 1),
                        )
                    nc.scalar.activation(h[:, m, :], ps[:, :], AF.Gelu,
                                         bias=b1[:, b, m:m + 1])
                return h

            def ffn2(i, h, split_out=False):
                b, n0, nt = sched[i]
                y = actp.tile([128, MD * nt], BF16, tag="y",
                              padded_shape=[128, MD * MAX_N])
                for md in range(MD):
                    ps = ps2.tile([128, nt], F32, tag="ps2",
                                  padded_shape=[128, MAX_N])
                    for k in range(K2):
                        nc.tensor.matmul(
                            ps[:, :],
                            w2[:, b, md, k, :],
                            h[:, k, :],
                            start=(k == 0),
                            stop=(k == K2 - 1),
                        )
                    nc.vector.tensor_copy(y[:, md * nt:(md + 1) * nt],
                                          ps[:, :])
                    if split_out and md == 2:
                        nc.sync.dma_start(yT_d[:, xoff[i]:xoff[i] + 3 * nt],
                                          y[:, :3 * nt])
                if split_out:
                    nc.scalar.dma_start(
                        yT_d[:, xoff[i] + 3 * nt:xoff[i] + MD * nt],
                        y[:, 3 * nt:MD * nt])
                else:
                    nc.gpsimd.dma_start(yT_d[:, xoff[i]:xoff[i] + MD * nt],
                                        y[:, :])

            # Software-pipelined emission: FFN1(t) ahead of FFN2(t-1) so
            # the PE never waits on the gelu of the tile it just produced.
            prev = None
            seen_blocks = 0
            pend = []
            for i in range(len(sched)):
                if i + 3 < len(sched):
                    xdma(i + 3)
                if i == 0:
                    wdma(1, 0)
                elif i == 1:
                    wdma(1, 1)
                elif i == 2:
                    wdma(2, 0)
                elif i == 3:
                    wdma(2, 1)
                if i > 3 and sched[i][0] != sched[i - 1][0]:
                    # First tile of a new block: request the weights for
                    # the block after next, w2 one tile later.
                    seen_blocks += 1
                    pend.append((seen_blocks + 2, 1))
                    wdma(seen_blocks + 2, 0)
                elif pend:
                    wdma(*pend.pop(0))
                h = ffn1(i)
                if prev is not None:
                    # Last two tiles: halves on the idle sync/scalar queues
                    # so the drain doesn't serialize on gpsimd.
                    ffn2(prev[0], prev[1],
                         split_out=(prev[0] >= len(sched) - 2))
                prev = (i, h)
            if prev is not None:
                ffn2(prev[0], prev[1], split_out=True)

    nc.compile()
    return nc


_PROGRAM_CACHE = {}


def _get_program(counts):
    key = tuple(counts)
    if key not in _PROGRAM_CACHE:
        _PROGRAM_CACHE[key] = build_program(counts)
    return _PROGRAM_CACHE[key]


def kernel(x, gate_w, gate_b, w1, b1, w2, b2):
    x = np.asarray(x)
    w1 = np.asarray(w1)
    b1 = np.asarray(b1)
    w2 = np.asarray(w2)
    b2 = np.asarray(b2)
    xt = x.reshape(T, D)

    # --- Routing on host (fp64; softmax is monotonic => argmax of logits) ---
    logits = xt.astype(np.float64) @ np.asarray(gate_w, np.float64)
    logits += np.asarray(gate_b, np.float64)
    eidx = np.argmax(logits, axis=-1)
    counts = np.bincount(eidx, minlength=E)

    # Schedule order: ascending token count (small first block => fast
    # ramp; the tail tile of the last block is made small separately).
    order = sorted(range(E), key=lambda e: counts[e])
    sched_counts = [int(counts[e]) for e in order]

    nc = _get_program(sched_counts)
    sched = make_sched(sched_counts)

    # --- Packed token buffer: per-tile [KD, nt] blocks, contiguous ---
    perm = np.concatenate([np.nonzero(eidx == e)[0] for e in order])
    xt_bf = xt.astype(ml_dtypes.bfloat16)[perm]          # [T, D]
    XL = KD * T
    xTg = np.empty((128, XL), ml_dtypes.bfloat16)
    for i, (_, n0, nt) in enumerate(sched):
        seg = xt_bf[n0:n0 + nt].T.reshape(KD, 128, nt).transpose(1, 0, 2)
        xTg[:, KD * n0:KD * (n0 + nt)] = seg.reshape(128, KD * nt)

    in_maps = []
    for q in range(N_CORES):
        w1q = np.empty((128, E, M1, KD, 128), ml_dtypes.bfloat16)
        w2q = np.empty((128, E, MD, K2, 128), ml_dtypes.bfloat16)
        b1q = np.empty((128, E, M1), np.float32)
        for bslot, e in enumerate(order):
            w1e = w1[e][:, q * FS:(q + 1) * FS]          # [D, FS]
            w1q[:, bslot] = w1e.reshape(KD, 128, M1, 128).transpose(
                1, 2, 0, 3).astype(ml_dtypes.bfloat16)
            w2e = w2[e][q * FS:(q + 1) * FS, :]          # [FS, D]
            w2q[:, bslot] = w2e.reshape(K2, 128, MD, 128).transpose(
                1, 2, 0, 3).astype(ml_dtypes.bfloat16)
            b1q[:, bslot] = b1[e][q * FS:(q + 1) * FS].reshape(M1, 128).T
        in_maps.append({"xT": xTg, "w1": w1q, "w2": w2q, "b1": b1q})

    res = bass_utils.run_bass_kernel_spmd(nc, in_maps,
                                          core_ids=list(range(N_CORES)),
                                          trace=TRACE)
    global LAST_RESULT
    LAST_RESULT = res

    acc = res.results[0]["yT"].astype(np.float32)
    for q in range(1, N_CORES):
        acc += res.results[q]["yT"].astype(np.float32)
    # unpack per-tile [MD, nt] segments -> [T, D]
    yg = np.empty((T, D), np.float32)
    for (_, n0, nt) in sched:
        seg = acc[:, KD * n0:KD * (n0 + nt)].reshape(128, MD, nt)
        yg[n0:n0 + nt] = seg.transpose(1, 0, 2).reshape(D, nt).T
    out = np.empty((T, D), np.float32)
    out[perm] = yg + b2[eidx[perm]]
    return out.reshape(B, S, D)


# revision 28
# speedup vs baseline: 1.0321x; 1.0321x over previous
"""MoE top-1 routing kernel for Trainium2 (8 NeuronCores).

Reference computation (B=8, S=1024, D=768, E=8, F=3072):
    gates = softmax(x @ gate_w + gate_b); expert_idx = argmax(gates)
    out[t] = gelu(x[t] @ w1[e] + b1[e]) @ w2[e] + b2[e]   for e = expert_idx[t]
    (no gate-probability scaling)

Strategy (v5 — feature-sliced expert replication, tile-packed IO):
  * Routing on host in fp64 (softmax is monotonic, so argmax of logits ==
    argmax of gates).
  * Every core holds ALL 8 experts' weights for its own 1/8 slice of the
    F dimension (384 features) and processes ALL T=8192 tokens, grouped
    into 8 expert blocks with EXACT token counts — zero padding, perfectly
    balanced across cores.  Each core emits a partial FFN2 sum; the host
    adds the 8 partials + b2.
  * Tokens stream through SBUF in tiles of <=512 (one PSUM bank).  x and
    y use a TILE-PACKED DRAM layout: tile i's [KD, nt] block is stored
    contiguously per partition, so each tile transfer is a single ~6KB
    descriptor per partition.  (DMA rings pay ~44ns fixed per descriptor;
    1KB-line layouts double total ring time and starve the PE.)
  * Weight loads are paced into the emission stream ~2 blocks ahead of
    use — ring FIFO means issue order is priority order.
  * Matmuls in bf16 with fp32 PSUM accumulation; activations stay
    transposed ([feature, token]).  gelu (erf) on the Scalar engine with
    the b1 bias fused; FFN2 partials copied PSUM->SBUF as bf16 on the
    Vector engine and DMA'd out per tile.
  * PE warmup matmuls flip the HAM clock gate to 2.4 GHz while the head
    DMAs stream in.
"""

import sys

try:
    import concourse  # noqa: F401
except ImportError:
    sys.path.insert(0, "/opt/trn_rl_repo")

import numpy as np
import ml_dtypes

import concourse.bass as bass  # noqa: F401
import concourse.tile as tile
import concourse.mybir as mybir
from concourse import bacc
from concourse import bass_utils

BF16 = mybir.dt.bfloat16
F32 = mybir.dt.float32
AF = mybir.ActivationFunctionType

B, S, D, E = 8, 1024, 768, 8
F = 4 * D           # 3072
T = B * S           # 8192
N_CORES = 8
FS = F // N_CORES   # 384 features per core
KD = D // 128       # 6 contraction chunks over D (FFN1)
M1 = FS // 128      # 3 output chunks over the F-slice (FFN1)
K2 = FS // 128      # 3 contraction chunks over the F-slice (FFN2)
MD = D // 128       # 6 output chunks over D (FFN2)
MAX_N = 512         # moving-dim tile (one fp32 PSUM bank)
N_WARMUP = 45       # PE warmup matmuls: ~4.8us cold > one HAM window,
                    # sized to end right as the first tile's DMAs land so
                    # the clock flips to 2.4GHz before the real stream.

# Debug/profiling knobs (used by the local test harness only).
TRACE = False
LAST_RESULT = None


def _even_split(cap):
    """ceil(cap/512) near-equal tiles."""
    if cap <= 0:
        return []
    n = -(-cap // MAX_N)
    base, rem = divmod(cap, n)
    out, off = [], 0
    for i in range(n):
        sz = base + (1 if i < rem else 0)
        out.append((off, sz))
        off += sz
    return out


def _ramp_split(cap):
    """Small leading tiles so the first matmuls need little DMA."""
    lead = [64, 192, 256]
    out, off = [], 0
    for w in lead:
        if cap - off <= w + MAX_N:
            break
        out.append((off, w))
        off += w
    return out + [(off + o, w) for (o, w) in _even_split(cap - off)]


def _tail_split(cap):
    """Equal tiles, then descending small final tiles so the trailing
    y-out DMAs overlap compute and the final drain is tiny."""
    tail = [288, 160, 96]
    if cap <= sum(tail) + MAX_N:
        return _even_split(cap)
    out = _even_split(cap - sum(tail))
    off = cap - sum(tail)
    for w in tail:
        out.append((off, w))
        off += w
    return out


def make_sched(counts):
    """counts: per-slot token counts (schedule order).  Returns
    [(slot, n0, nt)] tile schedule over the concatenated token buffer."""
    sched = []
    off = 0
    nb = len(counts)
    for b, c in enumerate(counts):
        if c == 0:
            continue
        if b == nb - 1:
            tiles = _tail_split(c)
        else:
            tiles = _even_split(c)
        for (o, w) in tiles:
            sched.append((b, off + o, w))
        off += c
    return sched


def build_program(counts):
    counts = list(counts)
    sched = make_sched(counts)
    XL = KD * T          # packed x/y length per partition (elements)

    nc = bacc.Bacc("TRN2", target_bir_lowering=False, debug=False,
                   num_devices=N_CORES)

    xT_d = nc.dram_tensor("xT", (128, XL), BF16, kind="ExternalInput")
    w1_d = nc.dram_tensor("w1", (128, E, M1, KD, 128), BF16,
                          kind="ExternalInput")
    w2_d = nc.dram_tensor("w2", (128, E, MD, K2, 128), BF16,
                          kind="ExternalInput")
    b1_d = nc.dram_tensor("b1", (128, E, M1), F32, kind="ExternalInput")
    yT_d = nc.dram_tensor("yT", (128, XL), BF16, kind="ExternalOutput")

    NXB = 5  # xT streaming buffers

    with tile.TileContext(nc) as tc:
        with (
            tc.tile_pool(name="wts", bufs=1) as wts,
            tc.tile_pool(name="xb", bufs=NXB) as xbp,
            tc.tile_pool(name="act", bufs=4) as actp,
            tc.tile_pool(name="ps1", bufs=3, space="PSUM") as ps1,
            tc.tile_pool(name="ps2", bufs=5, space="PSUM") as ps2,
        ):
            w1 = wts.tile([128, E, M1, KD, 128], BF16, tag="w1")
            w2 = wts.tile([128, E, MD, K2, 128], BF16, tag="w2")
            b1 = wts.tile([128, E, M1], F32, tag="b1")
            warm = wts.tile([128, 128], BF16, tag="warm")
            nc.gpsimd.memset(warm[:], 0.0)
            wps = ps1.tile([128, 128], F32, tag="ps1",
                           padded_shape=[128, MAX_N])

            # PE warmup: dummy matmuls run while the head DMAs stream in,
            # flipping the HAM clock gate to 2.4 GHz before the real
            # matmul stream starts.
            for _ in range(N_WARMUP):
                nc.tensor.matmul(wps[:, :], warm[:, :], warm[:, :])

            # ---- packed x tile streaming ----
            # Tile i's tokens live at xT_d[:, xoff[i] : xoff[i]+KD*nt]
            # (chunk k at sub-offset k*nt).  One descriptor per partition.
            xoff = []
            o = 0
            for (_, _, nt) in sched:
                xoff.append(o)
                o += KD * nt

            xtiles = {}

            def xdma(i):
                _, _, nt = sched[i]
                xt = xbp.tile([128, KD * nt], BF16, tag="x",
                              padded_shape=[128, KD * MAX_N])
                xtiles[i] = xt
                nc.sync.dma_start(xt[:, :], xT_d[:, xoff[i]:xoff[i] + KD * nt])

            # ---- head DMAs ----
            # Only what the ramp tiles need right away.  Issue order IS
            # ring priority, so nothing bulky goes ahead of the first
            # tiles' dependencies.  The scalar queue (gelu) gets only two
            # small issues; paced weight loads ride on gpsimd (the y-out
            # queue) where act bufs=3 gives two tiles of slack.
            b0 = sched[0][0]
            xdma(0)
            nc.scalar.dma_start(w1[:, b0, 0, :, :], w1_d[:, b0, 0, :, :])
            nc.scalar.dma_start(b1[:], b1_d[:])
            nc.gpsimd.dma_start(w1[:, b0, 1, :, :], w1_d[:, b0, 1, :, :])
            nc.gpsimd.dma_start(w1[:, b0, 2, :, :], w1_d[:, b0, 2, :, :])
            xdma(1)
            nc.gpsimd.dma_start(w2[:, b0, 0:3, :, :], w2_d[:, b0, 0:3, :, :])
            nc.gpsimd.dma_start(w2[:, b0, 3:, :, :], w2_d[:, b0, 3:, :, :])
            xdma(2)
            blocks = []
            for (b, _, _) in sched:
                if b not in blocks:
                    blocks.append(b)

            def wdma(bi, part):
                """Paced weight load for the bi-th block in schedule order.
                part 0 = w1, part 1 = w2 (staggered by one tile to smooth
                ring pressure)."""
                if bi < len(blocks):
                    b = blocks[bi]
                    if part == 0:
                        nc.gpsimd.dma_start(w1[:, b, :, :, :],
                                            w1_d[:, b, :, :, :])
                    else:
                        nc.gpsimd.dma_start(w2[:, b, :, :, :],
                                            w2_d[:, b, :, :, :])

            def ffn1(i):
                b, n0, nt = sched[i]
                xt = xtiles.pop(i)
                h = actp.tile([128, K2, nt], BF16, tag="h",
                              padded_shape=[128, K2, MAX_N])
                for m in range(M1):
                    ps = ps1.tile([128, nt], F32, tag="ps1",
                                  padded_shape=[128, MAX_N])
                    for k in range(KD):
                        nc.tensor.matmul(
                            ps[:, :],
                            w1[:, b, m, k, :],
                            xt[:, k * nt:(k + 1) * nt],
                            start=(k == 0),
                            stop=(k == KD - 1),
                        )
                    nc.scalar.activation(h[:, m, :], ps[:, :], AF.Gelu,
                                         bias=b1[:, b, m:m + 1])
                return h

            def ffn2(i, h, split_out=False):
                b, n0, nt = sched[i]
                y = actp.tile([128, MD * nt], BF16, tag="y",
                              padded_shape=[128, MD * MAX_N])
                for md in range(MD):
                    ps = ps2.tile([128, nt], F32, tag="ps2",
                                  padded_shape=[128, MAX_N])
                    for k in range(K2):
                        nc.tensor.matmul(
                            ps[:, :],
                            w2[:, b, md, k, :],
                            h[:, k, :],
                            start=(k == 0),
                            stop=(k == K2 - 1),
                        )
                    nc.vector.tensor_copy(y[:, md * nt:(md + 1) * nt],
                                          ps[:, :])
                    if split_out and md == 2:
                        nc.sync.dma_start(yT_d[:, xoff[i]:xoff[i] + 3 * nt],
                                          y[:, :3 * nt])
                if split_out:
                    nc.scalar.dma_start(
                        yT_d[:, xoff[i] + 3 * nt:xoff[i] + MD * nt],
                        y[:, 3 * nt:MD * nt])
                else:
                    nc.gpsimd.dma_start(yT_d[:, xoff[i]:xoff[i] + MD * nt],
                                        y[:, :])

            # Software-pipelined emission: FFN1(t) ahead of FFN2(t-1) so
            # the PE never waits on the gelu of the tile it just produced.
            prev = None
            seen_blocks = 0
            pend = []
            for i in range(len(sched)):
                if i + 3 < len(sched):
                    xdma(i + 3)
                if i == 0:
                    wdma(1, 0)
                elif i == 1:
                    wdma(1, 1)
                elif i == 2:
                    wdma(2, 0)
                elif i == 3:
                    wdma(2, 1)
                if i > 3 and sched[i][0] != sched[i - 1][0]:
                    # First tile of a new block: request the weights for
                    # the block after next, w2 one tile later.
                    seen_blocks += 1
                    pend.append((seen_blocks + 2, 1))
                    wdma(seen_blocks + 2, 0)
                elif pend:
                    wdma(*pend.pop(0))
                h = ffn1(i)
                if prev is not None:
                    # Last two tiles: halves on the idle sync/scalar queues
                    # so the drain doesn't serialize on gpsimd.
                    ffn2(prev[0], prev[1],
                         split_out=(prev[0] >= len(sched) - 2))
                prev = (i, h)
            if prev is not None:
                ffn2(prev[0], prev[1], split_out=True)

    nc.compile()
    return nc


_PROGRAM_CACHE = {}


def _get_program(counts):
    key = tuple(counts)
    if key not in _PROGRAM_CACHE:
        _PROGRAM_CACHE[key] = build_program(counts)
    return _PROGRAM_CACHE[key]


def kernel(x, gate_w, gate_b, w1, b1, w2, b2):
    x = np.asarray(x)
    w1 = np.asarray(w1)
    b1 = np.asarray(b1)
    w2 = np.asarray(w2)
    b2 = np.asarray(b2)
    xt = x.reshape(T, D)

    # --- Routing on host (fp64; softmax is monotonic => argmax of logits) ---
    logits = xt.astype(np.float64) @ np.asarray(gate_w, np.float64)
    logits += np.asarray(gate_b, np.float64)
    eidx = np.argmax(logits, axis=-1)
    counts = np.bincount(eidx, minlength=E)

    # Schedule order: ascending token count (small first block => fast
    # ramp; the tail tile of the last block is made small separately).
    order = sorted(range(E), key=lambda e: counts[e])
    sched_counts = [int(counts[e]) for e in order]

    nc = _get_program(sched_counts)
    sched = make_sched(sched_counts)

    # --- Packed token buffer: per-tile [KD, nt] blocks, contiguous ---
    perm = np.concatenate([np.nonzero(eidx == e)[0] for e in order])
    xt_bf = xt.astype(ml_dtypes.bfloat16)[perm]          # [T, D]
    XL = KD * T
    xTg = np.empty((128, XL), ml_dtypes.bfloat16)
    for i, (_, n0, nt) in enumerate(sched):
        seg = xt_bf[n0:n0 + nt].T.reshape(KD, 128, nt).transpose(1, 0, 2)
        xTg[:, KD * n0:KD * (n0 + nt)] = seg.reshape(128, KD * nt)

    in_maps = []
    for q in range(N_CORES):
        w1q = np.empty((128, E, M1, KD, 128), ml_dtypes.bfloat16)
        w2q = np.empty((128, E, MD, K2, 128), ml_dtypes.bfloat16)
        b1q = np.empty((128, E, M1), np.float32)
        for bslot, e in enumerate(order):
            w1e = w1[e][:, q * FS:(q + 1) * FS]          # [D, FS]
            w1q[:, bslot] = w1e.reshape(KD, 128, M1, 128).transpose(
                1, 2, 0, 3).astype(ml_dtypes.bfloat16)
            w2e = w2[e][q * FS:(q + 1) * FS, :]          # [FS, D]
            w2q[:, bslot] = w2e.reshape(K2, 128, MD, 128).transpose(
                1, 2, 0, 3).astype(ml_dtypes.bfloat16)
            b1q[:, bslot] = b1[e][q * FS:(q + 1) * FS].reshape(M1, 128).T
        in_maps.append({"xT": xTg, "w1": w1q, "w2": w2q, "b1": b1q})

    res = bass_utils.run_bass_kernel_spmd(nc, in_maps,
                                          core_ids=list(range(N_CORES)),
                                          trace=TRACE)
    global LAST_RESULT
    LAST_RESULT = res

    acc = res.results[0]["yT"].astype(np.float32)
    for q in range(1, N_CORES):
        acc += res.results[q]["yT"].astype(np.float32)
    # unpack per-tile [MD, nt] segments -> [T, D]
    yg = np.empty((T, D), np.float32)
    for (_, n0, nt) in sched:
        seg = acc[:, KD * n0:KD * (n0 + nt)].reshape(128, MD, nt)
        yg[n0:n0 + nt] = seg.transpose(1, 0, 2).reshape(D, nt).T
    out = np.empty((T, D), np.float32)
    out[perm] = yg + b2[eidx[perm]]
    return out.reshape(B, S, D)
